# revision 2
# baseline (speedup 1.0000x reference)
"""Trainium2 Bass kernel for ChamferkNNDist.

Problem (B=8, N=4096, 3-D points):
  chamfer = mean_b mean_j min_i ||adv[b,j] - ori[b,i]||^2
  knn: per adv point, mean of its 5 nearest-neighbour sq-distances within
       adv[b] (excluding self), then a mean+1.05*std threshold mask.
  out = 5*chamfer + 3*knn_loss  (fp32 scalar)

Strategy: data-parallel over batch, one batch element per NeuronCore (8 cores).
Per core both N x N squared-distance matrices are computed as augmented
matmuls producing -D directly:  -||x-y||^2 = [2x, -|x|^2, -1] . [y, 1, |y|^2]
(bf16 hi/lo split, K=15, for fp32-grade accuracy).

The baseline was VectorEngine-bound: the DVE had to scan every PSUM element
(MAX8 for kNN top-k, max-reduce for chamfer) at ~1 elem/cycle/lane @0.96GHz.
This version halves the DVE scan with a pairwise-max fold computed by the
TensorEngine + ScalarEngine:
    max(-D1, -D2) = -D2 + relu(D2 - D1)
  MM1: PSUM_A = -D2 = u . v2            (pair second-halves)
  MM2: PSUM_B = D2 - D1 = u . (v1 - v2) (host-precomputed diff operand)
  ACT: R = relu(PSUM_B) -> SBUF f32r    (ScalarE does this scan, not DVE)
  MM3: PSUM_A += I . R                  (f32r identity matmul accumulate)
  DVE: MAX8 / max-reduce over PSUM_A    (half the original width)
f32r keeps ~13 mantissa bits; simulated end-to-end rel err ~5e-5.

MM1/MM2 optionally run concurrently in two PE row-groups (K=15 each,
tile_position (0,0)/(32,0)) with operands staged at SBUF partitions 0-14 and
32-46. Tiny per-row results (top8 pairs [N,16], rowmax [N,2]) are DMA'd out
and the final mean/std/threshold reduction is done on host in fp64.
"""

import sys

if "/opt/trn_rl_repo" not in sys.path:
    sys.path.insert(0, "/opt/trn_rl_repo")

from contextlib import ExitStack

import numpy as np

import concourse.bacc as bacc
import concourse.tile as tile
from concourse import mybir
from concourse.bass_utils import run_bass_kernel_spmd

F32 = mybir.dt.float32
F32R = mybir.dt.float32r
BF16 = mybir.dt.bfloat16

B = 8
N = 4096
NCORES = 8
HALF = N // 2  # pair space width
CW = 1024  # chunk width in pair columns (2 PSUM banks for A, 2 for B)

CHAMFER_W = 5.0
KNN_W = 3.0
KNN_K = 5
KNN_ALPHA = 1.05

VARIANT = "fold"  # "fold" | "fold_flat" (no row-group tiling)


def build_program(n=N, reps=1, variant=None):
    """Bass program for one core: one batch element of size n.

    Inputs (host-prepared, bf16 hi/lo split K=15):
      ua2 [47, n]: rows 0-14 and 32-46 both hold the adv weight rows
          [2a; -|a|^2; -1] (split), per adv point (matmul stationary operand,
          staged twice for the two PE row-groups).
      vv [47, n/2]: rows 0-14 = va[:, n/2:] (adv moving operand second
          halves), rows 32-46 = va[:, :n/2] - va[:, n/2:] (pair diffs).
      ww [47, n/2]: same for ori (chamfer).
      idm [128, 128]: fp32 identity (MM3 stationary operand).
    Outputs:
      top8 [n, 16]: per row, 8 largest pair-maxes of -D_adv,adv per 1024-pair
          chunk (2 chunks, descending each).
      cmax [n, 2]: per row, max of folded -D_adv,ori per chunk.
    reps > 1 wraps the body in a hardware loop (for timing only).
    """
    variant = variant or VARIANT
    rowtile = variant == "fold"
    nt = n // 128
    nchunk = HALF // CW  # chunks per tile per matrix
    nc = bacc.Bacc("TRN2", target_bir_lowering=False, debug=False)
    ua2 = nc.dram_tensor("ua2", [47, n], BF16, kind="ExternalInput").ap()
    vv = nc.dram_tensor("vv", [47, HALF], BF16, kind="ExternalInput").ap()
    ww = nc.dram_tensor("ww", [47, HALF], BF16, kind="ExternalInput").ap()
    idm = nc.dram_tensor("idm", [128, 128], F32, kind="ExternalInput").ap()
    top8 = nc.dram_tensor("top8", [n, 8 * nchunk], F32, kind="ExternalOutput").ap()
    cmax = nc.dram_tensor("cmax", [n, nchunk], F32, kind="ExternalOutput").ap()

    with tile.TileContext(nc) as tc:
        with ExitStack() as ctx:
            const_pool = ctx.enter_context(tc.tile_pool(name="const", bufs=1))
            pa_pool = ctx.enter_context(
                tc.tile_pool(name="pa", bufs=2, space="PSUM")
            )
            pb_pool = ctx.enter_context(
                tc.tile_pool(name="pb", bufs=2, space="PSUM")
            )
            r_pool = ctx.enter_context(tc.tile_pool(name="r", bufs=3))
            out_pool = ctx.enter_context(tc.tile_pool(name="out", bufs=3))

            ua_s = const_pool.tile([47, n], BF16)
            nc.sync.dma_start(ua_s[:], ua2)
            vv_s = const_pool.tile([47, HALF], BF16)
            nc.sync.dma_start(vv_s[:], vv)
            ww_s = const_pool.tile([47, HALF], BF16)
            nc.sync.dma_start(ww_s[:], ww)
            id_f32 = const_pool.tile([128, 128], F32)
            nc.sync.dma_start(id_f32[:], idm)
            id_s = const_pool.tile([128, 128], F32R)
            nc.vector.tensor_copy(id_s[:], id_f32[:])

            tp0 = (0, 0) if rowtile else None
            tp1 = (32, 0) if rowtile else None

            def body(_i=None):
                for t in range(nt):
                    lhs0 = ua_s[0:15, t * 128 : (t + 1) * 128]
                    lhs1 = ua_s[32:47, t * 128 : (t + 1) * 128]
                    t8cat = out_pool.tile([128, 8 * nchunk], F32, tag="t8")
                    cm = out_pool.tile([128, nchunk], F32, tag="cm")
                    for mv_s, is_knn in ((vv_s, True), (ww_s, False)):
                        for c in range(nchunk):
                            lo = c * CW
                            pa = pa_pool.tile([128, CW], F32, tag="pa")
                            pb = pb_pool.tile([128, CW], F32, tag="pb")
                            for j in range(CW // 512):
                                sl = slice(j * 512, (j + 1) * 512)
                                gsl = slice(lo + j * 512, lo + (j + 1) * 512)
                                nc.tensor.matmul(
                                    pa[:, sl], lhs0, mv_s[0:15, gsl],
                                    start=True, stop=False,
                                    tile_position=tp0, skip_group_check=True,
                                )
                            for j in range(CW // 512):
                                sl = slice(j * 512, (j + 1) * 512)
                                gsl = slice(lo + j * 512, lo + (j + 1) * 512)
                                nc.tensor.matmul(
                                    pb[:, sl], lhs1, mv_s[32:47, gsl],
                                    start=True, stop=True,
                                    tile_position=tp1, skip_group_check=True,
                                )
                            r = r_pool.tile([128, CW], F32R, tag="r")
                            nc.scalar.activation(
                                r[:], pb[:], mybir.ActivationFunctionType.Relu
                            )
                            for j in range(CW // 512):
                                sl = slice(j * 512, (j + 1) * 512)
                                nc.tensor.matmul(
                                    pa[:, sl], id_s[:], r[:, sl],
                                    start=False, stop=True,
                                    skip_group_check=True,
                                )
                            if is_knn:
                                nc.vector.max(
                                    t8cat[:, c * 8 : (c + 1) * 8], pa[:]
                                )
                            else:
                                nc.vector.tensor_reduce(
                                    cm[:, c : c + 1], pa[:],
                                    axis=mybir.AxisListType.X,
                                    op=mybir.AluOpType.max,
                                )
                    nc.sync.dma_start(top8[t * 128 : (t + 1) * 128, :], t8cat[:])
                    nc.sync.dma_start(cmax[t * 128 : (t + 1) * 128, :], cm[:])

            if reps == 1:
                body()
            else:
                with tc.For_i(0, reps, 1):
                    body()
    nc.compile()
    return nc


def make_inputs(adv_pc, ori_pc, variant=None):
    """Per-core input dicts: augmented + pair-diff matmul operand matrices."""
    import ml_dtypes

    bf = ml_dtypes.bfloat16
    adv = np.asarray(adv_pc, dtype=np.float32)
    ori = np.asarray(ori_pc, dtype=np.float32)
    ident = np.eye(128, dtype=np.float32)

    def split15(m):
        hi = m.astype(bf)
        lo = (m - hi.astype(np.float32)).astype(bf)
        return np.concatenate([hi, hi, lo], 0)

    def split15u(m):
        hi = m.astype(bf)
        lo = (m - hi.astype(np.float32)).astype(bf)
        return np.concatenate([hi, lo, hi], 0)

    in_maps = []
    for b in range(B):
        a, o = adv[b], ori[b]
        na = (a * a).sum(1, dtype=np.float32)[None, :]
        no = (o * o).sum(1, dtype=np.float32)[None, :]
        one = np.ones((1, N), np.float32)
        ua = np.concatenate([2.0 * a.T, -na, -one], 0).astype(np.float32)
        va = np.concatenate([a.T, one, na], 0).astype(np.float32)
        vo = np.concatenate([o.T, one, no], 0).astype(np.float32)

        ua15 = split15u(ua)  # [15, N]
        ua47 = np.zeros((47, N), bf)
        ua47[0:15] = ua15
        ua47[32:47] = ua15

        def pack_mv(v):
            v2 = v[:, HALF:]
            vd = v[:, :HALF] - v[:, HALF:]
            m = np.zeros((47, HALF), bf)
            m[0:15] = split15(v2)
            m[32:47] = split15(vd)
            return m

        in_maps.append(
            {
                "ua2": ua47,
                "vv": pack_mv(va),
                "ww": pack_mv(vo),
                "idm": ident,
            }
        )
    return in_maps


def finalize(results):
    """Host-side (fp64) final reduction from per-core top8/cmax outputs."""
    loss1 = np.empty(B, np.float64)
    knn = np.empty(B, np.float64)
    for b in range(B):
        top8 = results[b]["top8"].astype(np.float64)  # [N, 16] of -D pairmax
        cmax = results[b]["cmax"].astype(np.float64)  # [N, 2] of max(-D)
        loss1[b] = (-cmax.max(axis=1)).mean()
        # merged top-6: rank 0 is the self pair (-D ~ 0); 1..5 are the 5-NN
        d6 = np.sort(-top8, axis=1)[:, : KNN_K + 1]
        value = d6[:, 1:].mean(axis=1)
        mean = value.mean()
        std = value.std(ddof=1)
        thresh = mean + KNN_ALPHA * std
        knn[b] = (value * (value > thresh)).mean()
    total = CHAMFER_W * loss1.mean() + KNN_W * knn.mean()
    return np.float32(total)


_program_cache = {}


def kernel(adv_pc, ori_pc):
    key = VARIANT
    if key not in _program_cache:
        _program_cache[key] = build_program()
    nc = _program_cache[key]
    in_maps = make_inputs(adv_pc, ori_pc)
    res = run_bass_kernel_spmd(nc, in_maps, core_ids=list(range(NCORES)))
    return finalize(res.results)


# revision 6
# speedup vs baseline: 1.2935x; 1.2935x over previous
"""Trainium2 Bass kernel for ChamferkNNDist.

Problem (B=8, N=4096, 3-D points):
  chamfer = mean_b mean_j min_i ||adv[b,j] - ori[b,i]||^2
  knn: per adv point, mean of its 5 nearest-neighbour sq-distances within
       adv[b] (excluding self), then a mean+1.05*std threshold mask.
  out = 5*chamfer + 3*knn_loss  (fp32 scalar)

Strategy: data-parallel over batch, one batch element per NeuronCore (8 cores).
Per core both N x N squared-distance matrices are computed as augmented
matmuls producing -D directly:  -||x-y||^2 = [2x, -|x|^2, -1] . [y, 1, |y|^2]
(bf16 hi/lo split, K=15, for fp32-grade accuracy).

The baseline was VectorEngine-bound: the DVE had to scan every PSUM element
(MAX8 for kNN top-k, max-reduce for chamfer) at ~1 elem/cycle/lane @0.96GHz.
This version halves the DVE scan with a pairwise-max fold computed by the
TensorEngine + ScalarEngine:
    max(-D1, -D2) = -D2 + relu(D2 - D1)
  MM1: PSUM_A = -D2 = u . v2            (pair second-halves)
  MM2: PSUM_B = D2 - D1 = u . (v1 - v2) (host-precomputed diff operand)
  ACT: R = relu(PSUM_B) -> SBUF f32r    (ScalarE does this scan, not DVE)
  MM3: PSUM_A += I . R                  (f32r identity matmul accumulate)
  DVE: MAX8 / max-reduce over PSUM_A    (half the original width)
f32r keeps ~13 mantissa bits; simulated end-to-end rel err ~5e-5.

MM1/MM2 optionally run concurrently in two PE row-groups (K=15 each,
tile_position (0,0)/(32,0)) with operands staged at SBUF partitions 0-14 and
32-46. Tiny per-row results (top8 pairs [N,16], rowmax [N,2]) are DMA'd out
and the final mean/std/threshold reduction is done on host in fp64.
"""

import sys

if "/opt/trn_rl_repo" not in sys.path:
    sys.path.insert(0, "/opt/trn_rl_repo")

from contextlib import ExitStack

import numpy as np

import concourse.bacc as bacc
import concourse.tile as tile
from concourse import mybir
from concourse.bass_utils import run_bass_kernel_spmd

F32 = mybir.dt.float32
F32R = mybir.dt.float32r
BF16 = mybir.dt.bfloat16

B = 8
N = 4096
NCORES = 8
HALF = N // 2  # pair space width
CW = 1024  # chunk width in pair columns (2 PSUM banks for A, 2 for B)

CHAMFER_W = 5.0
KNN_W = 3.0
KNN_K = 5
KNN_ALPHA = 1.05

VARIANT = "fold"  # "fold" | "fold_flat" (no row-group tiling)


def build_program(n=N, reps=1, variant=None):
    """Bass program for one core: one batch element of size n.

    Inputs (host-prepared, bf16 hi/lo split K=15):
      ua2 [47, n]: rows 0-14 and 32-46 both hold the adv weight rows
          [2a; -|a|^2; -1] (split), per adv point (matmul stationary operand,
          staged twice for the two PE row-groups).
      vv [47, n/2]: rows 0-14 = va[:, n/2:] (adv moving operand second
          halves), rows 32-46 = va[:, :n/2] - va[:, n/2:] (pair diffs).
      ww [47, n/2]: same for ori (chamfer).
      idm [128, 128]: fp32 identity (MM3 stationary operand).
    Outputs:
      top8 [n, 16]: per row, 8 largest pair-maxes of -D_adv,adv per 1024-pair
          chunk (2 chunks, descending each).
      cmax [n, 2]: per row, max of folded -D_adv,ori per chunk.
    reps > 1 wraps the body in a hardware loop (for timing only).
    """
    import os

    ablate = set(
        a for a in os.environ.get("KABLATE", "").split(",") if a
    )
    variant = variant or VARIANT
    rowtile = variant == "fold"
    nt = n // 128
    nchunk = HALF // CW  # chunks per tile per matrix
    nc = bacc.Bacc("TRN2", target_bir_lowering=False, debug=False)
    ua2 = nc.dram_tensor("ua2", [47, n], BF16, kind="ExternalInput").ap()
    vv = nc.dram_tensor("vv", [47, HALF], BF16, kind="ExternalInput").ap()
    ww = nc.dram_tensor("ww", [47, HALF], BF16, kind="ExternalInput").ap()
    idm = nc.dram_tensor("idm", [128, 128], F32, kind="ExternalInput").ap()
    top8 = nc.dram_tensor("top8", [n, 8 * nchunk], F32, kind="ExternalOutput").ap()
    cmax = nc.dram_tensor("cmax", [n, nchunk], F32, kind="ExternalOutput").ap()

    with tile.TileContext(nc) as tc:
        with ExitStack() as ctx:
            const_pool = ctx.enter_context(tc.tile_pool(name="const", bufs=1))
            pa_pool = ctx.enter_context(
                tc.tile_pool(name="pa", bufs=2, space="PSUM")
            )
            pb_pool = ctx.enter_context(
                tc.tile_pool(name="pb", bufs=2, space="PSUM")
            )
            r_pool = ctx.enter_context(tc.tile_pool(name="r", bufs=3))
            out_pool = ctx.enter_context(tc.tile_pool(name="out", bufs=3))

            ua_s = const_pool.tile([47, n], BF16)
            nc.sync.dma_start(ua_s[:], ua2)
            vv_s = const_pool.tile([47, HALF], BF16)
            nc.sync.dma_start(vv_s[:], vv)
            ww_s = const_pool.tile([47, HALF], BF16)
            nc.sync.dma_start(ww_s[:], ww)
            id_f32 = const_pool.tile([128, 128], F32)
            nc.sync.dma_start(id_f32[:], idm)
            id_s = const_pool.tile([128, 128], F32R)
            nc.vector.tensor_copy(id_s[:], id_f32[:])

            tp0 = (0, 0) if rowtile else None
            tp1 = (32, 0) if rowtile else None

            def body(_i=None):
                # software-pipelined: stage1 (MM1, MM2, ACT relu) of unit k
                # overlaps stage2 (MM3 accumulate, DVE scan) of unit k-1, so
                # the in-order PE queue never stalls waiting for the ACT.
                units = [
                    (t, is_knn, c)
                    for t in range(nt)
                    for is_knn in (True, False)
                    for c in range(nchunk)
                ]
                tiles_out = {}
                staged = {}

                def stage1(u):
                    t, is_knn, c = u
                    lhs0 = ua_s[0:15, t * 128 : (t + 1) * 128]
                    lhs1 = ua_s[32:47, t * 128 : (t + 1) * 128]
                    mv_s = vv_s if is_knn else ww_s
                    lo = c * CW
                    pa = pa_pool.tile([128, CW], F32, tag="pa")
                    pb = pb_pool.tile([128, CW], F32, tag="pb")
                    for j in range(CW // 512):
                        sl = slice(j * 512, (j + 1) * 512)
                        gsl = slice(lo + j * 512, lo + (j + 1) * 512)
                        nc.tensor.matmul(
                            pa[:, sl], lhs0, mv_s[0:15, gsl],
                            start=True, stop=False,
                            tile_position=tp0, skip_group_check=True,
                        )
                    for j in range(CW // 512):
                        sl = slice(j * 512, (j + 1) * 512)
                        gsl = slice(lo + j * 512, lo + (j + 1) * 512)
                        nc.tensor.matmul(
                            pb[:, sl], lhs1, mv_s[32:47, gsl],
                            start=True, stop=True,
                            tile_position=tp1, skip_group_check=True,
                        )
                    r = r_pool.tile([128, CW], F32R, tag="r")
                    nc.scalar.activation(
                        r[:], pb[:], mybir.ActivationFunctionType.Relu
                    )
                    staged[u] = (pa, r)

                def stage2(u):
                    t, is_knn, c = u
                    pa, r = staged.pop(u)
                    for j in range(CW // 512):
                        sl = slice(j * 512, (j + 1) * 512)
                        nc.tensor.matmul(
                            pa[:, sl], id_s[:], r[:, sl],
                            start=False, stop=True, skip_group_check=True,
                        )
                    if t not in tiles_out:
                        t8cat = out_pool.tile([128, 8 * nchunk], F32, tag="t8")
                        cm = out_pool.tile([128, nchunk], F32, tag="cm")
                        tiles_out[t] = (t8cat, cm)
                    t8cat, cm = tiles_out[t]
                    if is_knn:
                        nc.vector.max(t8cat[:, c * 8 : (c + 1) * 8], pa[:])
                    else:
                        nc.vector.tensor_reduce(
                            cm[:, c : c + 1], pa[:],
                            axis=mybir.AxisListType.X,
                            op=mybir.AluOpType.max,
                        )
                    if not is_knn and c == nchunk - 1:
                        del tiles_out[t]
                        nc.sync.dma_start(
                            top8[t * 128 : (t + 1) * 128, :], t8cat[:]
                        )
                        nc.sync.dma_start(
                            cmax[t * 128 : (t + 1) * 128, :], cm[:]
                        )

                prev = None
                for u in units:
                    stage1(u)
                    if prev is not None:
                        stage2(prev)
                    prev = u
                stage2(prev)

            if reps == 1:
                body()
            else:
                with tc.For_i(0, reps, 1):
                    body()
    nc.compile()
    return nc


def make_inputs(adv_pc, ori_pc, variant=None):
    """Per-core input dicts: augmented + pair-diff matmul operand matrices."""
    import ml_dtypes

    bf = ml_dtypes.bfloat16
    adv = np.asarray(adv_pc, dtype=np.float32)
    ori = np.asarray(ori_pc, dtype=np.float32)
    ident = np.eye(128, dtype=np.float32)

    def split15(m):
        hi = m.astype(bf)
        lo = (m - hi.astype(np.float32)).astype(bf)
        return np.concatenate([hi, hi, lo], 0)

    def split15u(m):
        hi = m.astype(bf)
        lo = (m - hi.astype(np.float32)).astype(bf)
        return np.concatenate([hi, lo, hi], 0)

    in_maps = []
    for b in range(B):
        a, o = adv[b], ori[b]
        na = (a * a).sum(1, dtype=np.float32)[None, :]
        no = (o * o).sum(1, dtype=np.float32)[None, :]
        one = np.ones((1, N), np.float32)
        ua = np.concatenate([2.0 * a.T, -na, -one], 0).astype(np.float32)
        va = np.concatenate([a.T, one, na], 0).astype(np.float32)
        vo = np.concatenate([o.T, one, no], 0).astype(np.float32)

        ua15 = split15u(ua)  # [15, N]
        ua47 = np.zeros((47, N), bf)
        ua47[0:15] = ua15
        ua47[32:47] = ua15

        def pack_mv(v):
            v2 = v[:, HALF:]
            vd = v[:, :HALF] - v[:, HALF:]
            m = np.zeros((47, HALF), bf)
            m[0:15] = split15(v2)
            m[32:47] = split15(vd)
            return m

        in_maps.append(
            {
                "ua2": ua47,
                "vv": pack_mv(va),
                "ww": pack_mv(vo),
                "idm": ident,
            }
        )
    return in_maps


def finalize(results):
    """Host-side (fp64) final reduction from per-core top8/cmax outputs."""
    loss1 = np.empty(B, np.float64)
    knn = np.empty(B, np.float64)
    for b in range(B):
        top8 = results[b]["top8"].astype(np.float64)  # [N, 16] of -D pairmax
        cmax = results[b]["cmax"].astype(np.float64)  # [N, 2] of max(-D)
        loss1[b] = (-cmax.max(axis=1)).mean()
        # merged top-6: rank 0 is the self pair (-D ~ 0); 1..5 are the 5-NN
        d6 = np.sort(-top8, axis=1)[:, : KNN_K + 1]
        value = d6[:, 1:].mean(axis=1)
        mean = value.mean()
        std = value.std(ddof=1)
        thresh = mean + KNN_ALPHA * std
        knn[b] = (value * (value > thresh)).mean()
    total = CHAMFER_W * loss1.mean() + KNN_W * knn.mean()
    return np.float32(total)


_program_cache = {}


def kernel(adv_pc, ori_pc):
    key = VARIANT
    if key not in _program_cache:
        _program_cache[key] = build_program()
    nc = _program_cache[key]
    in_maps = make_inputs(adv_pc, ori_pc)
    res = run_bass_kernel_spmd(nc, in_maps, core_ids=list(range(NCORES)))
    return finalize(res.results)


# revision 9
# speedup vs baseline: 1.6284x; 1.2589x over previous
"""Trainium2 Bass kernel for ChamferkNNDist.

Problem (B=8, N=4096, 3-D points):
  chamfer = mean_b mean_j min_i ||adv[b,j] - ori[b,i]||^2
  knn: per adv point, mean of its 5 nearest-neighbour sq-distances within
       adv[b] (excluding self), then a mean+1.05*std threshold mask.
  out = 5*chamfer + 3*knn_loss  (fp32 scalar)

Strategy: data-parallel over batch, one batch element per NeuronCore (8 cores).
Per core both N x N squared-distance matrices are computed as augmented
matmuls producing -D directly:  -||x-y||^2 = [2x, -|x|^2, -1] . [y, 1, |y|^2]
(bf16 hi/lo split, K=15, for fp32-grade accuracy).

The baseline was VectorEngine-bound: the DVE had to scan every PSUM element
(MAX8 for kNN top-k, max-reduce for chamfer) at ~1 elem/cycle/lane @0.96GHz.
This version halves the DVE scan with a pairwise-max fold computed by the
TensorEngine + ScalarEngine:
    max(-D1, -D2) = -D2 + relu(D2 - D1)
  MM1: PSUM_A = -D2 = u . v2            (pair second-halves)
  MM2: PSUM_B = D2 - D1 = u . (v1 - v2) (host-precomputed diff operand)
  ACT: R = relu(PSUM_B) -> SBUF f32r    (ScalarE does this scan, not DVE)
  MM3: PSUM_A += I . R                  (f32r identity matmul accumulate)
  DVE: MAX8 / max-reduce over PSUM_A    (half the original width)
f32r keeps ~13 mantissa bits; simulated end-to-end rel err ~5e-5.

MM1/MM2 optionally run concurrently in two PE row-groups (K=15 each,
tile_position (0,0)/(32,0)) with operands staged at SBUF partitions 0-14 and
32-46. Tiny per-row results (top8 pairs [N,16], rowmax [N,2]) are DMA'd out
and the final mean/std/threshold reduction is done on host in fp64.
"""

import sys

if "/opt/trn_rl_repo" not in sys.path:
    sys.path.insert(0, "/opt/trn_rl_repo")

from contextlib import ExitStack

import numpy as np

import concourse.bacc as bacc
import concourse.tile as tile
from concourse import mybir
from concourse.bass_utils import run_bass_kernel_spmd

F32 = mybir.dt.float32
F32R = mybir.dt.float32r
BF16 = mybir.dt.bfloat16

B = 8
N = 4096
NCORES = 8
HALF = N // 2  # pair space width
CW = 1024  # chunk width in pair columns (2 PSUM banks for A, 2 for B)

CHAMFER_W = 5.0
KNN_W = 3.0
KNN_K = 5
KNN_ALPHA = 1.05

VARIANT = "fold"  # "fold" | "fold_flat" (no row-group tiling)


def build_program(n=N, reps=1, variant=None):
    """Bass program for one core: one batch element of size n.

    Inputs (host-prepared, bf16 hi/lo split K=15):
      ua2 [47, n]: rows 0-14 and 32-46 both hold the adv weight rows
          [2a; -|a|^2; -1] (split), per adv point (matmul stationary operand,
          staged twice for the two PE row-groups).
      vv [47, n/2]: rows 0-14 = va[:, n/2:] (adv moving operand second
          halves), rows 32-46 = va[:, :n/2] - va[:, n/2:] (pair diffs).
      ww [47, n/2]: same for ori (chamfer).
      idm [128, 128]: fp32 identity (MM3 stationary operand).
    Outputs:
      top8 [n, 16]: per row, 8 largest pair-maxes of -D_adv,adv per 1024-pair
          chunk (2 chunks, descending each).
      cmax [n, 2]: per row, max of folded -D_adv,ori per chunk.
    reps > 1 wraps the body in a hardware loop (for timing only).
    """
    import os

    ablate = set(
        a for a in os.environ.get("KABLATE", "").split(",") if a
    )
    variant = variant or VARIANT
    rowtile = variant == "fold"
    nt = n // 128
    nchunk = HALF // CW  # chunks per tile per matrix
    nc = bacc.Bacc("TRN2", target_bir_lowering=False, debug=False)
    ua2 = nc.dram_tensor("ua2", [47, n], BF16, kind="ExternalInput").ap()
    vv = nc.dram_tensor("vv", [47, HALF], BF16, kind="ExternalInput").ap()
    ww = nc.dram_tensor("ww", [47, HALF], BF16, kind="ExternalInput").ap()
    idm = nc.dram_tensor("idm", [128, 128], F32, kind="ExternalInput").ap()
    top8 = nc.dram_tensor("top8", [n, 8 * nchunk], F32, kind="ExternalOutput").ap()
    cmax = nc.dram_tensor("cmax", [n, nchunk], F32, kind="ExternalOutput").ap()

    with tile.TileContext(nc) as tc:
        with ExitStack() as ctx:
            const_pool = ctx.enter_context(tc.tile_pool(name="const", bufs=1))
            pa_pool = ctx.enter_context(
                tc.tile_pool(name="pa", bufs=3, space="PSUM")
            )
            pb_pool = ctx.enter_context(
                tc.tile_pool(name="pb", bufs=2, space="PSUM")
            )
            r_pool = ctx.enter_context(tc.tile_pool(name="r", bufs=4))
            out_pool = ctx.enter_context(tc.tile_pool(name="out", bufs=3))

            ua_s = const_pool.tile([47, n], BF16)
            nc.sync.dma_start(ua_s[:], ua2)
            vv_s = const_pool.tile([47, HALF], BF16)
            nc.sync.dma_start(vv_s[:], vv)
            ww_s = const_pool.tile([47, HALF], BF16)
            nc.sync.dma_start(ww_s[:], ww)
            id_f32 = const_pool.tile([128, 128], F32)
            nc.sync.dma_start(id_f32[:], idm)
            id_s = const_pool.tile([128, 128], F32R)
            nc.vector.tensor_copy(id_s[:], id_f32[:])

            tp0 = (0, 0) if rowtile else None
            tp1 = (32, 0) if rowtile else None

            def body(_i=None):
                # software-pipelined: stage1 (MM1, MM2, ACT relu) of unit k
                # overlaps stage2 (MM3 accumulate, DVE scan) of unit k-1, so
                # the in-order PE queue never stalls waiting for the ACT.
                units = [
                    (t, is_knn, c)
                    for t in range(nt)
                    for is_knn in (True, False)
                    for c in range(nchunk)
                ]
                tiles_out = {}
                staged = {}

                def stage1(u):
                    t, is_knn, c = u
                    lhs0 = ua_s[0:15, t * 128 : (t + 1) * 128]
                    lhs1 = ua_s[32:47, t * 128 : (t + 1) * 128]
                    mv_s = vv_s if is_knn else ww_s
                    lo = c * CW
                    pa = pa_pool.tile([128, CW], F32, tag="pa")
                    r = r_pool.tile([128, CW], F32R, tag="r")
                    # 512-wide (single PSUM bank) diff sub-chunks so the relu
                    # can start as early as possible and pb stays at 2 banks
                    for j in range(CW // 512):
                        sl = slice(j * 512, (j + 1) * 512)
                        gsl = slice(lo + j * 512, lo + (j + 1) * 512)
                        nc.tensor.matmul(
                            pa[:, sl], lhs0, mv_s[0:15, gsl],
                            start=True, stop=False,
                            tile_position=tp0, skip_group_check=True,
                        )
                        pb = pb_pool.tile([128, 512], F32, tag="pb")
                        nc.tensor.matmul(
                            pb[:], lhs1, mv_s[32:47, gsl],
                            start=True, stop=True,
                            tile_position=tp1, skip_group_check=True,
                        )
                        nc.scalar.activation(
                            r[:, sl], pb[:], mybir.ActivationFunctionType.Relu
                        )
                    staged[u] = (pa, r)

                def stage2(u):
                    t, is_knn, c = u
                    pa, r = staged.pop(u)
                    for j in range(CW // 512):
                        sl = slice(j * 512, (j + 1) * 512)
                        nc.tensor.matmul(
                            pa[:, sl], id_s[:], r[:, sl],
                            start=False, stop=True, skip_group_check=True,
                        )
                    if t not in tiles_out:
                        t8cat = out_pool.tile([128, 8 * nchunk], F32, tag="t8")
                        cm = out_pool.tile([128, nchunk], F32, tag="cm")
                        tiles_out[t] = (t8cat, cm)
                    t8cat, cm = tiles_out[t]
                    if is_knn:
                        nc.vector.max(t8cat[:, c * 8 : (c + 1) * 8], pa[:])
                    else:
                        nc.vector.tensor_reduce(
                            cm[:, c : c + 1], pa[:],
                            axis=mybir.AxisListType.X,
                            op=mybir.AluOpType.max,
                        )
                    if not is_knn and c == nchunk - 1:
                        del tiles_out[t]
                        nc.sync.dma_start(
                            top8[t * 128 : (t + 1) * 128, :], t8cat[:]
                        )
                        nc.sync.dma_start(
                            cmax[t * 128 : (t + 1) * 128, :], cm[:]
                        )

                DELAY = 2
                for i, u in enumerate(units):
                    stage1(u)
                    if i >= DELAY:
                        stage2(units[i - DELAY])
                for u in units[-DELAY:]:
                    stage2(u)

            if reps == 1:
                body()
            else:
                with tc.For_i(0, reps, 1):
                    body()
    nc.compile()
    return nc


def make_inputs(adv_pc, ori_pc, variant=None):
    """Per-core input dicts: augmented + pair-diff matmul operand matrices."""
    import ml_dtypes

    bf = ml_dtypes.bfloat16
    adv = np.asarray(adv_pc, dtype=np.float32)
    ori = np.asarray(ori_pc, dtype=np.float32)
    ident = np.eye(128, dtype=np.float32)

    def split15(m):
        hi = m.astype(bf)
        lo = (m - hi.astype(np.float32)).astype(bf)
        return np.concatenate([hi, hi, lo], 0)

    def split15u(m):
        hi = m.astype(bf)
        lo = (m - hi.astype(np.float32)).astype(bf)
        return np.concatenate([hi, lo, hi], 0)

    in_maps = []
    for b in range(B):
        a, o = adv[b], ori[b]
        na = (a * a).sum(1, dtype=np.float32)[None, :]
        no = (o * o).sum(1, dtype=np.float32)[None, :]
        one = np.ones((1, N), np.float32)
        ua = np.concatenate([2.0 * a.T, -na, -one], 0).astype(np.float32)
        va = np.concatenate([a.T, one, na], 0).astype(np.float32)
        vo = np.concatenate([o.T, one, no], 0).astype(np.float32)

        ua15 = split15u(ua)  # [15, N]
        ua47 = np.zeros((47, N), bf)
        ua47[0:15] = ua15
        ua47[32:47] = ua15

        def pack_mv(v):
            v2 = v[:, HALF:]
            vd = v[:, :HALF] - v[:, HALF:]
            m = np.zeros((47, HALF), bf)
            m[0:15] = split15(v2)
            m[32:47] = split15(vd)
            return m

        in_maps.append(
            {
                "ua2": ua47,
                "vv": pack_mv(va),
                "ww": pack_mv(vo),
                "idm": ident,
            }
        )
    return in_maps


def finalize(results):
    """Host-side (fp64) final reduction from per-core top8/cmax outputs."""
    loss1 = np.empty(B, np.float64)
    knn = np.empty(B, np.float64)
    for b in range(B):
        top8 = results[b]["top8"].astype(np.float64)  # [N, 16] of -D pairmax
        cmax = results[b]["cmax"].astype(np.float64)  # [N, 2] of max(-D)
        loss1[b] = (-cmax.max(axis=1)).mean()
        # merged top-6: rank 0 is the self pair (-D ~ 0); 1..5 are the 5-NN
        d6 = np.sort(-top8, axis=1)[:, : KNN_K + 1]
        value = d6[:, 1:].mean(axis=1)
        mean = value.mean()
        std = value.std(ddof=1)
        thresh = mean + KNN_ALPHA * std
        knn[b] = (value * (value > thresh)).mean()
    total = CHAMFER_W * loss1.mean() + KNN_W * knn.mean()
    return np.float32(total)


_program_cache = {}


def kernel(adv_pc, ori_pc):
    key = VARIANT
    if key not in _program_cache:
        _program_cache[key] = build_program()
    nc = _program_cache[key]
    in_maps = make_inputs(adv_pc, ori_pc)
    res = run_bass_kernel_spmd(nc, in_maps, core_ids=list(range(NCORES)))
    return finalize(res.results)
